# revision 1
# baseline (speedup 1.0000x reference)
"""Trainium2 Bass kernel: 3x3 valid conv (64ch -> 128ch) + per-pixel bias.

Strategy: shard the 510 output rows spatially across 8 NeuronCores (64
rows/core with a 2-row input halo; core 7 overlaps core 6 by 2 rows).
Inside a core, the 64-row band is split across the two PE row-strips:
partitions 0-63 hold the input rows for output rows 0-31 of the band,
partitions 64-127 the rows for output rows 32-63.  Each output row is
9 accumulating K=64 float32r matmuls (one per kernel tap, N=510); the
two strips run concurrently, so a tap-pair costs one N=510 stream.
Bias is added during PSUM evacuation on the Vector engine.

float32r streams at 1 cycle/row (vs 4 for fp32) and keeps 11 explicit
mantissa bits; operands are pre-rounded on the host (the HW requires
fp32r-rounded inputs), so the result error vs the fp32 reference is
only the ~2^-13 input-rounding noise.
"""

import numpy as np
from contextlib import ExitStack

import concourse.bass as bass
import concourse.tile as tile
from concourse import bacc, mybir
from concourse import bass_utils

C, H, W = 64, 512, 512
D, KK = 128, 3
OH, OW = H - KK + 1, W - KK + 1          # 510, 510
NCORES = 8
RPC = 64                                  # output rows per core
BAND = RPC + KK - 1                       # 66 input rows per core
HALF = RPC // 2                           # 32 output rows per strip
IBAND = HALF + KK - 1                     # 34 input rows per strip
GROUPS = 8
GROWS = HALF // GROUPS                    # 4 pair-rows per group

f32 = mybir.dt.float32
f32r = mybir.dt.float32r

# row offset of each core's output band
STARTS = [min(i * RPC, OH - RPC) for i in range(NCORES)]

_CACHE = {}

# results of the last hardware run (inspected by test harnesses)
LAST_RESULTS = None


def _build_program():
    nc = bacc.Bacc(
        "TRN2", target_bir_lowering=False, debug=False, num_devices=NCORES
    )
    x = nc.dram_tensor("x", [C, BAND, W], f32r, kind="ExternalInput").ap()
    w = nc.dram_tensor("w", [C, 9 * D], f32r, kind="ExternalInput").ap()
    b = nc.dram_tensor("b", [D, RPC, OW], f32, kind="ExternalInput").ap()
    y = nc.dram_tensor("y", [D, RPC, OW], f32, kind="ExternalOutput").ap()

    x_flat = x.rearrange("c r w -> c (r w)")
    b_flat = b.rearrange("d r x -> d (r x)")
    y_flat = y.rearrange("d r x -> d (r x)")

    with tile.TileContext(nc) as tc:
        with ExitStack() as ctx:
            xp = ctx.enter_context(tc.tile_pool(name="xin", bufs=1))
            wp = ctx.enter_context(tc.tile_pool(name="wt", bufs=1))
            bp = ctx.enter_context(tc.tile_pool(name="bias", bufs=3))
            op = ctx.enter_context(tc.tile_pool(name="out", bufs=3))
            pp = ctx.enter_context(tc.tile_pool(name="ps", bufs=4, space="PSUM"))

            wt = wp.tile([128, 9 * D], f32r)
            nc.sync.dma_start(wt[0:64, :], w[:, :])
            nc.sync.dma_start(wt[64:128, :], w[:, :])

            # input band, both strips; chunked loads so compute starts early
            xin = xp.tile([128, IBAND * W], f32r)
            bounds = [0, 10, 18, 26, IBAND]
            for ci in range(len(bounds) - 1):
                r0, r1 = bounds[ci], bounds[ci + 1]
                nc.sync.dma_start(
                    xin[0:64, r0 * W:r1 * W], x_flat[:, r0 * W:r1 * W]
                )
                nc.sync.dma_start(
                    xin[64:128, r0 * W:r1 * W],
                    x_flat[:, (HALF + r0) * W:(HALF + r1) * W],
                )

            for g in range(GROUPS):
                ra = g * GROWS                 # band rows ra..ra+3  (strip 0)
                rb = HALF + ra                 # band rows rb..rb+3  (strip 1)
                ba = bp.tile([128, GROWS * OW], f32, tag="ba")
                nc.sync.dma_start(ba[:], b_flat[:, ra * OW:(ra + GROWS) * OW])
                bb = bp.tile([128, GROWS * OW], f32, tag="bb")
                nc.sync.dma_start(bb[:], b_flat[:, rb * OW:(rb + GROWS) * OW])
                ya = op.tile([128, GROWS * OW], f32, tag="ya")
                yb = op.tile([128, GROWS * OW], f32, tag="yb")

                for j in range(GROWS):
                    yl = ra + j                # strip-local output row
                    pa = pp.tile([128, OW], f32, tag="pa")
                    pb = pp.tile([128, OW], f32, tag="pb")
                    for t in range(9):
                        ky, kx = divmod(t, 3)
                        off = (yl + ky) * W + kx
                        nc.tensor.matmul(
                            pa[:],
                            wt[0:64, t * D:(t + 1) * D],
                            xin[0:64, off:off + OW],
                            start=(t == 0), stop=(t == 8),
                        )
                        nc.tensor.matmul(
                            pb[:],
                            wt[64:128, t * D:(t + 1) * D],
                            xin[64:128, off:off + OW],
                            start=(t == 0), stop=(t == 8),
                        )
                    nc.vector.tensor_add(
                        ya[:, j * OW:(j + 1) * OW], pa[:], ba[:, j * OW:(j + 1) * OW]
                    )
                    nc.vector.tensor_add(
                        yb[:, j * OW:(j + 1) * OW], pb[:], bb[:, j * OW:(j + 1) * OW]
                    )

                nc.sync.dma_start(y_flat[:, ra * OW:(ra + GROWS) * OW], ya[:])
                nc.sync.dma_start(y_flat[:, rb * OW:(rb + GROWS) * OW], yb[:])

    nc.compile()
    return nc


def _round_fp32r(a):
    """Round-to-nearest-even onto the fp32r grid (low 12 mantissa bits zero)."""
    u = np.ascontiguousarray(a, dtype=np.float32).view(np.uint32)
    u2 = (u.astype(np.uint64) + 0x7FF + ((u >> 12) & 1)) & 0xFFFFF000
    return u2.astype(np.uint32).view(np.float32)


def kernel(input, kernels, biases):
    global LAST_RESULTS
    if "nc" not in _CACHE:
        _CACHE["nc"] = _build_program()
    nc = _CACHE["nc"]

    xr = _round_fp32r(input)                                   # [C, H, W]
    wr = _round_fp32r(
        np.ascontiguousarray(kernels.transpose(1, 2, 3, 0)).reshape(C, 9 * D)
    )
    biases = np.ascontiguousarray(biases, dtype=np.float32)

    in_maps = []
    for s in STARTS:
        in_maps.append({
            "x": np.ascontiguousarray(xr[:, s:s + BAND, :]),
            "w": wr,
            "b": np.ascontiguousarray(biases[:, s:s + RPC, :]),
        })

    res = bass_utils.run_bass_kernel_spmd(
        nc, in_maps, core_ids=list(range(NCORES))
    )
    LAST_RESULTS = res

    out = np.empty((D, OH, OW), np.float32)
    for i, s in enumerate(STARTS):
        out[:, s:s + RPC, :] = res.results[i]["y"]
    return out


# revision 2
# speedup vs baseline: 1.1770x; 1.1770x over previous
"""Trainium2 Bass kernel: 3x3 valid conv (64ch -> 128ch) + per-pixel bias.

Strategy: shard the 510 output rows spatially across 8 NeuronCores (64
rows/core with a 2-row input halo; core 7 overlaps core 6 by 2 rows).
Inside a core, the 64-row band is split across the two PE row-strips:
partitions 0-63 hold the input rows for output rows 0-31 of the band,
partitions 64-127 the rows for output rows 32-63 (the host feeds the
band pre-split so every DMA runs at full 128-partition width).  Each
output row is 9 accumulating K=64 float32r matmuls (one per kernel
tap, N=510); the two strips run concurrently, so a tap-pair costs one
N=510 stream.  Bias is added during PSUM evacuation on the Vector
engine.  Loads and stores are spread across the scalar/sync/gpsimd DMA
paths to keep more descriptors in flight.

float32r streams at 1 cycle/row (vs 4 for fp32) and keeps 11 explicit
mantissa bits; operands are pre-rounded on the host (the HW requires
fp32r-rounded inputs), so the result error vs the fp32 reference is
only the ~2^-13 input-rounding noise.
"""

import numpy as np
from contextlib import ExitStack

import concourse.bass as bass
import concourse.tile as tile
from concourse import bacc, mybir
from concourse import bass_utils

C, H, W = 64, 512, 512
D, KK = 128, 3
OH, OW = H - KK + 1, W - KK + 1          # 510, 510
NCORES = 8
RPC = 64                                  # output rows per core
BAND = RPC + KK - 1                       # 66 input rows per core
HALF = RPC // 2                           # 32 output rows per strip
IBAND = HALF + KK - 1                     # 34 input rows per strip
GROUPS = 8
GROWS = HALF // GROUPS                    # 4 pair-rows per group

f32 = mybir.dt.float32
f32r = mybir.dt.float32r

# row offset of each core's output band
STARTS = [min(i * RPC, OH - RPC) for i in range(NCORES)]

_CACHE = {}

# results of the last hardware run (inspected by test harnesses)
LAST_RESULTS = None


def _build_program():
    nc = bacc.Bacc(
        "TRN2", target_bir_lowering=False, debug=False, num_devices=NCORES
    )
    # x is pre-split on the host: row (h*64+c) holds band rows
    # [32h, 32h+34) of channel c, flattened
    x = nc.dram_tensor("x", [2 * C, IBAND * W], f32r, kind="ExternalInput").ap()
    # w is pre-duplicated: rows 0-63 and 64-127 identical, [c, (ky kx d)]
    w = nc.dram_tensor("w", [2 * C, 9 * D], f32r, kind="ExternalInput").ap()
    b = nc.dram_tensor("b", [D, RPC, OW], f32, kind="ExternalInput").ap()
    y = nc.dram_tensor("y", [D, RPC, OW], f32, kind="ExternalOutput").ap()

    b_flat = b.rearrange("d r x -> d (r x)")
    y_flat = y.rearrange("d r x -> d (r x)")

    with tile.TileContext(nc) as tc:
        with ExitStack() as ctx:
            xp = ctx.enter_context(tc.tile_pool(name="xin", bufs=1))
            wp = ctx.enter_context(tc.tile_pool(name="wt", bufs=1))
            bp = ctx.enter_context(tc.tile_pool(name="bias", bufs=3))
            op = ctx.enter_context(tc.tile_pool(name="out", bufs=3))
            pp = ctx.enter_context(tc.tile_pool(name="ps", bufs=4, space="PSUM"))

            wt = wp.tile([128, 9 * D], f32r)
            nc.scalar.dma_start(wt[:], w[:, :])

            # input band, both strips; chunked loads so compute starts early
            xin = xp.tile([128, IBAND * W], f32r)
            bounds = [0, 6, 12, 18, 26, IBAND]
            for ci in range(len(bounds) - 1):
                r0, r1 = bounds[ci], bounds[ci + 1]
                nc.scalar.dma_start(
                    xin[:, r0 * W:r1 * W], x[:, r0 * W:r1 * W]
                )

            for g in range(GROUPS):
                ra = g * GROWS                 # band rows ra..ra+3  (strip 0)
                rb = HALF + ra                 # band rows rb..rb+3  (strip 1)
                ba = bp.tile([128, GROWS * OW], f32, tag="ba")
                nc.sync.dma_start(ba[:], b_flat[:, ra * OW:(ra + GROWS) * OW])
                bb = bp.tile([128, GROWS * OW], f32, tag="bb")
                nc.sync.dma_start(bb[:], b_flat[:, rb * OW:(rb + GROWS) * OW])
                ya = op.tile([128, GROWS * OW], f32, tag="ya")
                yb = op.tile([128, GROWS * OW], f32, tag="yb")

                for j in range(GROWS):
                    yl = ra + j                # strip-local output row
                    pa = pp.tile([128, OW], f32, tag="pa")
                    pb = pp.tile([128, OW], f32, tag="pb")
                    for t in range(9):
                        ky, kx = divmod(t, 3)
                        off = (yl + ky) * W + kx
                        nc.tensor.matmul(
                            pa[:],
                            wt[0:64, t * D:(t + 1) * D],
                            xin[0:64, off:off + OW],
                            start=(t == 0), stop=(t == 8),
                        )
                        nc.tensor.matmul(
                            pb[:],
                            wt[64:128, t * D:(t + 1) * D],
                            xin[64:128, off:off + OW],
                            start=(t == 0), stop=(t == 8),
                        )
                    nc.vector.tensor_add(
                        ya[:, j * OW:(j + 1) * OW], pa[:], ba[:, j * OW:(j + 1) * OW]
                    )
                    nc.vector.tensor_add(
                        yb[:, j * OW:(j + 1) * OW], pb[:], bb[:, j * OW:(j + 1) * OW]
                    )
                    # store per completed pair-of-rows to smooth the tail
                    if j % 2 == 1:
                        c0, c1 = (j - 1) * OW, (j + 1) * OW
                        nc.gpsimd.dma_start(
                            y_flat[:, (ra + j - 1) * OW:(ra + j + 1) * OW],
                            ya[:, c0:c1],
                        )
                        nc.gpsimd.dma_start(
                            y_flat[:, (rb + j - 1) * OW:(rb + j + 1) * OW],
                            yb[:, c0:c1],
                        )

    nc.compile()
    return nc


def _round_fp32r(a):
    """Round-to-nearest-even onto the fp32r grid (low 12 mantissa bits zero)."""
    u = np.ascontiguousarray(a, dtype=np.float32).view(np.uint32)
    u2 = (u.astype(np.uint64) + 0x7FF + ((u >> 12) & 1)) & 0xFFFFF000
    return u2.astype(np.uint32).view(np.float32)


def kernel(input, kernels, biases):
    global LAST_RESULTS
    if "nc" not in _CACHE:
        _CACHE["nc"] = _build_program()
    nc = _CACHE["nc"]

    xr = _round_fp32r(input)                                   # [C, H, W]
    w1 = _round_fp32r(
        np.ascontiguousarray(kernels.transpose(1, 2, 3, 0)).reshape(C, 9 * D)
    )
    wr = np.concatenate([w1, w1], axis=0)                      # [128, 9*D]
    biases = np.ascontiguousarray(biases, dtype=np.float32)

    in_maps = []
    for s in STARTS:
        band = xr[:, s:s + BAND, :]
        xs = np.concatenate(
            [band[:, 0:IBAND, :], band[:, HALF:HALF + IBAND, :]], axis=0
        ).reshape(2 * C, IBAND * W)
        in_maps.append({
            "x": np.ascontiguousarray(xs),
            "w": wr,
            "b": np.ascontiguousarray(biases[:, s:s + RPC, :]),
        })

    res = bass_utils.run_bass_kernel_spmd(
        nc, in_maps, core_ids=list(range(NCORES))
    )
    LAST_RESULTS = res

    out = np.empty((D, OH, OW), np.float32)
    for i, s in enumerate(STARTS):
        out[:, s:s + RPC, :] = res.results[i]["y"]
    return out
